# revision 24
# baseline (speedup 1.0000x reference)
"""Trainium2 Bass kernel for D2VEncoder.mask_tokens (span masking).

Reference semantics (B=16, T=4096, D=768, L=10, p=0.065):
    start[b,t]  = rand_vals[b,t] < p
    masked[b,t] = OR_{k=0..9} start[b, t-k]          (within the batch row)
    x_out       = where(masked[:, :, None], mask_token, x)
    mask        = where(masked, -inf, 0.0)

Sharding: data-parallel over batch, 2 rows per core on 8 NeuronCores; the
mask expansion is elementwise per (b,t) so no collectives are needed.

Per-core dataflow (memory-bound problem; ~49% of (b,t) rows are masked and
their x values are never needed):
  1. Window-OR over start indicators in chunk-major [64,128] layout on DVE
     (9 shifted adds with a 9-element halo loaded by a second small DMA).
  2. Predicate transposed to row-per-partition layout via a DRAM round-trip
     (DMA APs over DRAM are freely affine).
  3. x is streamed in 16 tiles of 512 rows. Each indirect-DMA index gathers
     TWO consecutive rows (6 KB); pairs with both rows masked get an
     out-of-bounds offset, which the DGE silently skips — cutting ~45% of
     read traffic. copy_predicated overwrites every masked row with
     mask_token, which also covers skipped pairs' stale SBUF slots.
     The first DENSE_HEAD tiles are read densely so streaming starts while
     the predicate chain is still computing.
  4. The 0/-inf attention mask is produced from the same predicate.

This compile path (bass2jax / walrus custom-kernel pipeline) accepts at most
ONE sync wait per instruction; _legalize_waits splits Tile's multi-wait
instructions into EventSemaphore + instruction, which is semantically
identical (the sequencer blocks on them in order).
"""

import numpy as np

import concourse.bass as bass
import concourse.mybir as mybir
from concourse.tile import TileContext, add_dep_helper
from concourse.bass_utils import run_bass_kernel_spmd

F32 = mybir.dt.float32
I32 = mybir.dt.int32

B, T, D = 16, 4096, 768
N_CORES = 8
B_LOC = B // N_CORES            # 2 batch rows per core
ROWS = B_LOC * T                # 8192 (b,t) rows per core
P = 128                         # partitions
N_CHUNK = ROWS // P             # 64 chunks of 128 rows
CH_PER_B = T // P               # 32 chunks per batch row
N_CHUNK2 = ROWS // (2 * P)      # 32 blocks of 256 rows (gather granularity)
L = 10                          # span length
HALO = L - 1
P_MASKING = 0.065

K2 = 2                          # 256-row blocks per streamed mega-tile
N_MEGA = N_CHUNK2 // K2         # 16 tiles of 512 rows
STREAM_BUFS = 6
DENSE_HEAD = 2                  # leading tiles read dense (no offsets dep)

_nc_cache = {}


def _legalize_waits(nc: bass.Bass) -> bass.Bass:
    """Split multi-wait instructions into EventSemaphore + instruction."""
    ctr = 0
    for f in nc.m.functions:
        for blk in f.blocks:
            out = []
            for ins in blk.instructions:
                si = ins.sync_info
                if si is not None and si.on_wait and len(si.on_wait) > 1:
                    waits = list(si.on_wait)
                    for w in waits[:-1]:
                        evs = mybir.InstEventSemaphore(ins=[], outs=[])
                        evs.name = f"I-waitsplit-{ctr}"
                        ctr += 1
                        evs.engine = ins.engine
                        evs.sync_info = mybir.SyncInfo(on_wait=[w], on_update=[])
                        out.append(evs)
                    ins.sync_info = mybir.SyncInfo(
                        on_wait=[waits[-1]], on_update=list(si.on_update)
                    )
                out.append(ins)
            blk.instructions[:] = out
    return nc


def _build_nc() -> bass.Bass:
    nc = bass.Bass()
    x = nc.declare_dram_parameter("x", [ROWS, D], F32, isOutput=False)
    rv = nc.declare_dram_parameter("rand_vals", [ROWS], F32, isOutput=False)
    tok = nc.declare_dram_parameter("mask_token", [D], F32, isOutput=False)
    iota2 = nc.declare_dram_parameter("iota2", [P, N_CHUNK2], I32, isOutput=False)
    xo = nc.declare_dram_parameter("x_out", [ROWS, D], F32, isOutput=True)
    mo = nc.declare_dram_parameter("mask_out", [ROWS], F32, isOutput=True)

    with TileContext(nc) as tc:
        with (
            tc.tile_pool(name="const", bufs=1) as cpool,
            tc.tile_pool(name="small", bufs=1) as spool,
            tc.tile_pool(name="dram", bufs=1, space="DRAM") as dpool,
            tc.tile_pool(name="stream", bufs=STREAM_BUFS) as stpool,
        ):
            # mask_token replicated to all 128 partitions
            tokt = cpool.tile([P, D], F32)
            nc.sync.dma_start(tokt[:], tok[None, :].broadcast_to([P, D]))
            # gate: make DVE observe the tokt DMA once so later consumers
            # need no extra wait
            gate_a = spool.tile([1, 1], F32)
            nc.vector.tensor_copy(gate_a[:], tokt[0:1, 0:1])

            # pair-base row indices iota2[p, c2] = c2*256 + 2p
            iott = cpool.tile([P, N_CHUNK2], I32)
            nc.sync.dma_start(iott[:], iota2[:, :])

            # rand values, chunk-major [64, 128]. Prep DMAs ride the scalar
            # (ACT) HWDGE ring so they don't queue behind the stream DMAs.
            rmain = spool.tile([N_CHUNK, P], F32)
            nc.scalar.dma_start(rmain[:], rv[:].rearrange("(c p) -> c p", p=P))
            # 9-element halo: chunk c gets rand[c*128-9 : c*128] (partitions
            # 1..63; rows 0 and 32 are batch-row starts -> no halo)
            rprev = spool.tile([N_CHUNK, HALO], F32)
            nc.scalar.dma_start(
                rprev[1:N_CHUNK, :],
                bass.AP(rv, P - HALO, [[P, N_CHUNK - 1], [1, HALO]]),
            )
            # overwrite the cross-batch-row halo (and chunk 0) with 1.0 (>= p)
            nc.vector.memset(rprev[0:1, :], 1.0)
            for b in range(1, B_LOC):
                nc.vector.memset(rprev[b * CH_PER_B : b * CH_PER_B + 1, :], 1.0)

            # halo'd start indicators, then the 10-wide window sum as a log
            # tree (10 = 8+2): 4 adds instead of 9 serial ones.
            # s01 col 9+j holds start[j] for j in [-9, 127].
            s01 = spool.tile([N_CHUNK, HALO + P], F32)
            nc.vector.tensor_scalar(
                s01[:, 0:HALO], rprev[:], P_MASKING, None, mybir.AluOpType.is_lt
            )
            nc.vector.tensor_scalar(
                s01[:, HALO:], rmain[:], P_MASKING, None, mybir.AluOpType.is_lt
            )
            t2 = spool.tile([N_CHUNK, P + 8], F32)   # col c: j=c-8, window {j, j-1}
            nc.vector.tensor_tensor(
                t2[:], s01[:, 1:], s01[:, 0 : P + 8], mybir.AluOpType.add
            )
            t4 = spool.tile([N_CHUNK, P + 6], F32)   # col c: j=c-6, window 4
            nc.vector.tensor_tensor(
                t4[:], t2[:, 2:], t2[:, 0 : P + 6], mybir.AluOpType.add
            )
            t8 = spool.tile([N_CHUNK, P + 2], F32)   # col c: j=c-2, window 8
            nc.vector.tensor_tensor(
                t8[:], t4[:, 4:], t4[:, 0 : P + 2], mybir.AluOpType.add
            )
            acc = spool.tile([N_CHUNK, P], F32)      # window {j..j-7} + {j-8, j-9}
            nc.vector.tensor_tensor(
                acc[:], t8[:, 2:], t2[:, 0:P], mybir.AluOpType.add
            )
            # integer 0/1 predicate (CopyPredicated requires an int mask)
            acc_i = spool.tile([N_CHUNK, P], I32)
            nc.vector.tensor_scalar(
                acc_i[:], acc[:], 0.0, None, mybir.AluOpType.is_gt
            )

            # pair-AND computed chunk-major BEFORE the round-trip so the
            # gather-critical transposed read is half the bytes and issues
            # first on the scalar ring: pairc[c, u] = pred[c*128+2u] AND +1
            pairc = spool.tile([N_CHUNK, P // 2], I32)
            a3 = acc_i[:].rearrange("c (u e) -> c u e", e=2)
            nc.vector.tensor_tensor(
                pairc[:], a3[:, :, 0], a3[:, :, 1], mybir.AluOpType.mult
            )
            pairD = dpool.tile([ROWS // 2], I32)
            nc.scalar.dma_start(
                pairD[:].rearrange("(c u) -> c u", u=P // 2), pairc[:]
            )
            # pair2T[p, c2] = pairc flat[c2*128 + p]  (pair at rows c2*256+2p)
            pair2T = spool.tile([P, N_CHUNK2], I32)
            nc.scalar.dma_start(
                pair2T[:],
                bass.AP(pairD[:].tensor, pairD[:].offset, [[1, P], [P, N_CHUNK2]]),
            )
            offs2 = spool.tile([P, N_CHUNK2], I32)
            nc.vector.tensor_scalar(
                offs2[:], pair2T[:], ROWS, None, mybir.AluOpType.mult
            )
            offs_done = nc.vector.tensor_tensor(
                offs2[:], offs2[:], iott[:], mybir.AluOpType.add
            )

            # full per-row predicate, pair-transposed, for the CP masks
            # (off the gather critical path):
            # pred2T[p, 2*c2+e] = pred[row c2*256 + 2p + e]
            predD = dpool.tile([ROWS], I32)
            nc.scalar.dma_start(predD[:].rearrange("(c p) -> c p", p=P), acc_i[:])
            pred2T = spool.tile([P, N_CHUNK], I32)
            nc.scalar.dma_start(
                pred2T[:],
                bass.AP(
                    predD[:].tensor,
                    predD[:].offset,
                    [[2, P], [2 * P, N_CHUNK2], [1, 2]],
                ),
            )

            # attention-mask output: 0 kept / -inf masked
            mval = spool.tile([N_CHUNK, P], F32)
            minf = spool.tile([N_CHUNK, P], F32)
            nc.vector.memset(mval[:], 0.0)
            nc.vector.memset(minf[:], float("-inf"))
            nc.vector.copy_predicated(mval[:], acc_i[:], minf[:])
            nc.sync.dma_start(mo[:].rearrange("(c p) -> c p", p=P), mval[:])

            # stream x: gather pairs with >=1 unmasked row, then overwrite
            # all masked rows with the token (covers skipped pairs too).
            # One shared bounds register (a to_reg per gather would exhaust
            # the gpsimd register file).
            bc_reg = nc.gpsimd.to_reg(ROWS - 1)
            for m in range(N_MEGA):
                tile = stpool.tile([P, K2 * 2 * D], F32)
                if m < DENSE_HEAD:
                    src = x[m * 2 * P * K2 : (m + 1) * 2 * P * K2, :].rearrange(
                        "(q p e) d -> p q e d", q=K2, e=2
                    )
                    nc.sync.dma_start(
                        tile[:].rearrange("p (q e d) -> p q e d", q=K2, e=2), src
                    )
                else:
                    for q in range(K2):
                        g = m * K2 + q
                        nc.gpsimd.indirect_dma_start(
                            out=tile[:, q * 2 * D : (q + 1) * 2 * D],
                            out_offset=None,
                            in_=x[:],
                            in_offset=bass.IndirectOffsetOnAxis(
                                ap=offs2[:, g : g + 1], axis=0
                            ),
                            bounds_check=bc_reg,
                            oob_is_err=False,
                        )
                # per-row selects: fine granularity pipelines best (each CP
                # starts right after its own half-tile's gather; a single
                # merged 3D-broadcast CP measured slower per element AND
                # serialized behind both gathers)
                for q in range(K2):
                    g = m * K2 + q
                    for e in range(2):
                        mask_ap = pred2T[:, 2 * g + e : 2 * g + e + 1].broadcast_to(
                            [P, D]
                        )
                        cp = nc.vector.copy_predicated(
                            tile[:, (q * 2 + e) * D : (q * 2 + e + 1) * D],
                            mask_ap,
                            tokt[:],
                        )
                        if m < DENSE_HEAD:
                            # keep the offsets chain ahead of these selects in
                            # DVE order: the first gather's wait otherwise
                            # trails the dense-head CPs by ~9us
                            add_dep_helper(
                                cp.ins,
                                offs_done.ins,
                                sync=False,
                                reason="offsets chain before dense-head CPs",
                            )
                dst = xo[m * 2 * P * K2 : (m + 1) * 2 * P * K2, :].rearrange(
                    "(q p e) d -> p q e d", q=K2, e=2
                )
                tv = tile[:].rearrange("p (q e d) -> p q e d", q=K2, e=2)
                nc.sync.dma_start(dst, tv)

    return _legalize_waits(nc)


def _get_nc() -> bass.Bass:
    if "nc" not in _nc_cache:
        _nc_cache["nc"] = _build_nc()
    return _nc_cache["nc"]


def _iota2() -> np.ndarray:
    p = np.arange(P, dtype=np.int32)[:, None]
    c2 = np.arange(N_CHUNK2, dtype=np.int32)[None, :]
    return np.ascontiguousarray(c2 * (2 * P) + 2 * p)


def kernel(**inputs) -> tuple[np.ndarray, np.ndarray]:
    x = np.ascontiguousarray(np.asarray(inputs["x"], dtype=np.float32))
    tok = np.ascontiguousarray(np.asarray(inputs["mask_token"], dtype=np.float32))
    rv = np.ascontiguousarray(np.asarray(inputs["rand_vals"], dtype=np.float32))
    iota = _iota2()

    in_maps = [
        {
            "x": x[i * B_LOC : (i + 1) * B_LOC].reshape(ROWS, D),
            "rand_vals": rv[i * B_LOC : (i + 1) * B_LOC].reshape(ROWS),
            "mask_token": tok,
            "iota2": iota,
        }
        for i in range(N_CORES)
    ]
    res = run_bass_kernel_spmd(_get_nc(), in_maps, list(range(N_CORES)))
    x_out = np.concatenate(
        [res.results[i]["x_out"].reshape(B_LOC, T, D) for i in range(N_CORES)], axis=0
    )
    mask = np.concatenate(
        [res.results[i]["mask_out"].reshape(B_LOC, T) for i in range(N_CORES)], axis=0
    )
    return x_out, mask


# revision 28
# speedup vs baseline: 1.0295x; 1.0295x over previous
"""Trainium2 Bass kernel for D2VEncoder.mask_tokens (span masking).

Reference semantics (B=16, T=4096, D=768, L=10, p=0.065):
    start[b,t]  = rand_vals[b,t] < p
    masked[b,t] = OR_{k=0..9} start[b, t-k]          (within the batch row)
    x_out       = where(masked[:, :, None], mask_token, x)
    mask        = where(masked, -inf, 0.0)

Sharding: data-parallel over batch, 2 rows per core on 8 NeuronCores; the
mask expansion is elementwise per (b,t) so no collectives are needed.

Per-core dataflow (memory-bound problem; ~49% of (b,t) rows are masked and
their x values are never needed):
  1. Window-OR over start indicators in chunk-major [64,128] layout on DVE
     (9 shifted adds with a 9-element halo loaded by a second small DMA).
  2. Predicate transposed to row-per-partition layout via a DRAM round-trip
     (DMA APs over DRAM are freely affine).
  3. x is streamed in 16 tiles of 512 rows. Each indirect-DMA index gathers
     TWO consecutive rows (6 KB); pairs with both rows masked get an
     out-of-bounds offset, which the DGE silently skips — cutting ~45% of
     read traffic. copy_predicated overwrites every masked row with
     mask_token, which also covers skipped pairs' stale SBUF slots.
     The first DENSE_HEAD tiles are read densely so streaming starts while
     the predicate chain is still computing.
  4. The 0/-inf attention mask is produced from the same predicate.

This compile path (bass2jax / walrus custom-kernel pipeline) accepts at most
ONE sync wait per instruction; _legalize_waits splits Tile's multi-wait
instructions into EventSemaphore + instruction, which is semantically
identical (the sequencer blocks on them in order).
"""

import numpy as np

import concourse.bass as bass
import concourse.mybir as mybir
from concourse.tile import TileContext, add_dep_helper
from concourse.bass_utils import run_bass_kernel_spmd

F32 = mybir.dt.float32
I32 = mybir.dt.int32

B, T, D = 16, 4096, 768
N_CORES = 8
B_LOC = B // N_CORES            # 2 batch rows per core
ROWS = B_LOC * T                # 8192 (b,t) rows per core
P = 128                         # partitions
N_CHUNK = ROWS // P             # 64 chunks of 128 rows
CH_PER_B = T // P               # 32 chunks per batch row
N_CHUNK2 = ROWS // (2 * P)      # 32 blocks of 256 rows (gather granularity)
L = 10                          # span length
HALO = L - 1
P_MASKING = 0.065

K2 = 2                          # 256-row blocks per streamed mega-tile
N_MEGA = N_CHUNK2 // K2         # 16 tiles of 512 rows
STREAM_BUFS = 6
DENSE_HEAD = 2                  # leading tiles read dense (no offsets dep)

_nc_cache = {}


def _legalize_waits(nc: bass.Bass) -> bass.Bass:
    """Split multi-wait instructions into EventSemaphore + instruction."""
    ctr = 0
    for f in nc.m.functions:
        for blk in f.blocks:
            out = []
            for ins in blk.instructions:
                si = ins.sync_info
                if si is not None and si.on_wait and len(si.on_wait) > 1:
                    waits = list(si.on_wait)
                    for w in waits[:-1]:
                        evs = mybir.InstEventSemaphore(ins=[], outs=[])
                        evs.name = f"I-waitsplit-{ctr}"
                        ctr += 1
                        evs.engine = ins.engine
                        evs.sync_info = mybir.SyncInfo(on_wait=[w], on_update=[])
                        out.append(evs)
                    ins.sync_info = mybir.SyncInfo(
                        on_wait=[waits[-1]], on_update=list(si.on_update)
                    )
                out.append(ins)
            blk.instructions[:] = out
    return nc


def _build_nc() -> bass.Bass:
    nc = bass.Bass()
    x = nc.declare_dram_parameter("x", [ROWS, D], F32, isOutput=False)
    rv = nc.declare_dram_parameter("rand_vals", [ROWS], F32, isOutput=False)
    tok = nc.declare_dram_parameter("mask_token", [D], F32, isOutput=False)
    iota2 = nc.declare_dram_parameter("iota2", [P, N_CHUNK2], I32, isOutput=False)
    xo = nc.declare_dram_parameter("x_out", [ROWS, D], F32, isOutput=True)
    mo = nc.declare_dram_parameter("mask_out", [ROWS], F32, isOutput=True)

    with TileContext(nc) as tc:
        with (
            tc.tile_pool(name="const", bufs=1) as cpool,
            tc.tile_pool(name="small", bufs=1) as spool,
            tc.tile_pool(name="dram", bufs=1, space="DRAM") as dpool,
            tc.tile_pool(name="stream", bufs=STREAM_BUFS) as stpool,
        ):
            # mask_token replicated to all 128 partitions
            tokt = cpool.tile([P, D], F32)
            nc.sync.dma_start(tokt[:], tok[None, :].broadcast_to([P, D]))
            # gate: make DVE observe the tokt DMA once so later consumers
            # need no extra wait
            gate_a = spool.tile([1, 1], F32)
            nc.vector.tensor_copy(gate_a[:], tokt[0:1, 0:1])

            # pair-base row indices iota2[p, c2] = c2*256 + 2p
            iott = cpool.tile([P, N_CHUNK2], I32)
            nc.sync.dma_start(iott[:], iota2[:, :])

            # rand values, chunk-major [64, 128]. Prep DMAs ride the scalar
            # (ACT) HWDGE ring so they don't queue behind the stream DMAs.
            rmain = spool.tile([N_CHUNK, P], F32)
            nc.scalar.dma_start(rmain[:], rv[:].rearrange("(c p) -> c p", p=P))
            # 9-element halo: chunk c gets rand[c*128-9 : c*128] (partitions
            # 1..63; rows 0 and 32 are batch-row starts -> no halo)
            rprev = spool.tile([N_CHUNK, HALO], F32)
            nc.scalar.dma_start(
                rprev[1:N_CHUNK, :],
                bass.AP(rv, P - HALO, [[P, N_CHUNK - 1], [1, HALO]]),
            )
            # overwrite the cross-batch-row halo (and chunk 0) with 1.0 (>= p)
            nc.vector.memset(rprev[0:1, :], 1.0)
            for b in range(1, B_LOC):
                nc.vector.memset(rprev[b * CH_PER_B : b * CH_PER_B + 1, :], 1.0)

            # halo'd start indicators, then the 10-wide window sum as a log
            # tree (10 = 8+2): 4 adds instead of 9 serial ones.
            # s01 col 9+j holds start[j] for j in [-9, 127].
            s01 = spool.tile([N_CHUNK, HALO + P], F32)
            nc.vector.tensor_scalar(
                s01[:, 0:HALO], rprev[:], P_MASKING, None, mybir.AluOpType.is_lt
            )
            nc.vector.tensor_scalar(
                s01[:, HALO:], rmain[:], P_MASKING, None, mybir.AluOpType.is_lt
            )
            t2 = spool.tile([N_CHUNK, P + 8], F32)   # col c: j=c-8, window {j, j-1}
            nc.vector.tensor_tensor(
                t2[:], s01[:, 1:], s01[:, 0 : P + 8], mybir.AluOpType.add
            )
            t4 = spool.tile([N_CHUNK, P + 6], F32)   # col c: j=c-6, window 4
            nc.vector.tensor_tensor(
                t4[:], t2[:, 2:], t2[:, 0 : P + 6], mybir.AluOpType.add
            )
            t8 = spool.tile([N_CHUNK, P + 2], F32)   # col c: j=c-2, window 8
            nc.vector.tensor_tensor(
                t8[:], t4[:, 4:], t4[:, 0 : P + 2], mybir.AluOpType.add
            )
            acc = spool.tile([N_CHUNK, P], F32)      # window {j..j-7} + {j-8, j-9}
            nc.vector.tensor_tensor(
                acc[:], t8[:, 2:], t2[:, 0:P], mybir.AluOpType.add
            )
            # integer 0/1 predicate (CopyPredicated requires an int mask)
            acc_i = spool.tile([N_CHUNK, P], I32)
            nc.vector.tensor_scalar(
                acc_i[:], acc[:], 0.0, None, mybir.AluOpType.is_gt
            )

            # pair-AND computed chunk-major BEFORE the round-trip so the
            # gather-critical transposed read is half the bytes and issues
            # first on the scalar ring: pairc[c, u] = pred[c*128+2u] AND +1
            pairc = spool.tile([N_CHUNK, P // 2], I32)
            a3 = acc_i[:].rearrange("c (u e) -> c u e", e=2)
            nc.vector.tensor_tensor(
                pairc[:], a3[:, :, 0], a3[:, :, 1], mybir.AluOpType.mult
            )
            pairD = dpool.tile([ROWS // 2], I32)
            nc.scalar.dma_start(
                pairD[:].rearrange("(c u) -> c u", u=P // 2), pairc[:]
            )
            # pair2T[p, c2] = pairc flat[c2*128 + p]  (pair at rows c2*256+2p)
            pair2T = spool.tile([P, N_CHUNK2], I32)
            nc.scalar.dma_start(
                pair2T[:],
                bass.AP(pairD[:].tensor, pairD[:].offset, [[1, P], [P, N_CHUNK2]]),
            )
            offs2 = spool.tile([P, N_CHUNK2], I32)
            nc.vector.tensor_scalar(
                offs2[:], pair2T[:], ROWS, None, mybir.AluOpType.mult
            )
            offs_done = nc.vector.tensor_tensor(
                offs2[:], offs2[:], iott[:], mybir.AluOpType.add
            )

            # full per-row predicate, pair-transposed, for the CP masks
            # (off the gather critical path):
            # pred2T[p, 2*c2+e] = pred[row c2*256 + 2p + e]
            # Round-tripped in two halves so the first tiles' masks arrive
            # a few us earlier (first CP gates the first x_out write).
            predD = dpool.tile([ROWS], I32)
            pred2T = spool.tile([P, N_CHUNK], I32)
            predC = predD[:].rearrange("(c p) -> c p", p=P)
            for h in range(2):
                ch = N_CHUNK // 2
                nc.scalar.dma_start(
                    predC[h * ch : (h + 1) * ch, :],
                    acc_i[h * ch : (h + 1) * ch, :],
                )
                nc.scalar.dma_start(
                    pred2T[:, h * ch : (h + 1) * ch],
                    bass.AP(
                        predD[:].tensor,
                        predD[:].offset + h * (ROWS // 2),
                        [[2, P], [2 * P, N_CHUNK2 // 2], [1, 2]],
                    ),
                )

            # attention-mask output: 0 kept / -inf masked
            mval = spool.tile([N_CHUNK, P], F32)
            minf = spool.tile([N_CHUNK, P], F32)
            nc.vector.memset(mval[:], 0.0)
            nc.vector.memset(minf[:], float("-inf"))
            nc.vector.copy_predicated(mval[:], acc_i[:], minf[:])
            nc.sync.dma_start(mo[:].rearrange("(c p) -> c p", p=P), mval[:])

            # stream x: gather pairs with >=1 unmasked row, then overwrite
            # all masked rows with the token (covers skipped pairs too).
            # One shared bounds register (a to_reg per gather would exhaust
            # the gpsimd register file). The last mega-tile is split in two
            # so the final (pipeline-tail) store is half as long.
            bc_reg = nc.gpsimd.to_reg(ROWS - 1)
            blocks = [(i * K2, K2) for i in range(N_MEGA - 1)]
            blocks += [((N_MEGA - 1) * K2 + j, 1) for j in range(K2)]
            for c2base, kk in blocks:
                tile = stpool.tile([P, K2 * 2 * D], F32)
                if c2base < DENSE_HEAD * K2:
                    src = x[c2base * 2 * P : (c2base + kk) * 2 * P, :].rearrange(
                        "(q p e) d -> p q e d", q=kk, e=2
                    )
                    nc.sync.dma_start(
                        tile[:, : kk * 2 * D].rearrange(
                            "p (q e d) -> p q e d", q=kk, e=2
                        ),
                        src,
                    )
                else:
                    for q in range(kk):
                        g = c2base + q
                        nc.gpsimd.indirect_dma_start(
                            out=tile[:, q * 2 * D : (q + 1) * 2 * D],
                            out_offset=None,
                            in_=x[:],
                            in_offset=bass.IndirectOffsetOnAxis(
                                ap=offs2[:, g : g + 1], axis=0
                            ),
                            bounds_check=bc_reg,
                            oob_is_err=False,
                        )
                # per-row selects: fine granularity pipelines best (each CP
                # starts right after its own half-tile's gather; a single
                # merged 3D-broadcast CP measured slower per element AND
                # serialized behind both gathers)
                for q in range(kk):
                    g = c2base + q
                    for e in range(2):
                        mask_ap = pred2T[:, 2 * g + e : 2 * g + e + 1].broadcast_to(
                            [P, D]
                        )
                        cp = nc.vector.copy_predicated(
                            tile[:, (q * 2 + e) * D : (q * 2 + e + 1) * D],
                            mask_ap,
                            tokt[:],
                        )
                        if c2base < DENSE_HEAD * K2:
                            # keep the offsets chain ahead of these selects in
                            # DVE order: the first gather's wait otherwise
                            # trails the dense-head CPs by ~9us
                            add_dep_helper(
                                cp.ins,
                                offs_done.ins,
                                sync=False,
                                reason="offsets chain before dense-head CPs",
                            )
                dst = xo[c2base * 2 * P : (c2base + kk) * 2 * P, :].rearrange(
                    "(q p e) d -> p q e d", q=kk, e=2
                )
                tv = tile[:, : kk * 2 * D].rearrange(
                    "p (q e d) -> p q e d", q=kk, e=2
                )
                nc.sync.dma_start(dst, tv)

    return _legalize_waits(nc)


def _get_nc() -> bass.Bass:
    if "nc" not in _nc_cache:
        _nc_cache["nc"] = _build_nc()
    return _nc_cache["nc"]


def _iota2() -> np.ndarray:
    p = np.arange(P, dtype=np.int32)[:, None]
    c2 = np.arange(N_CHUNK2, dtype=np.int32)[None, :]
    return np.ascontiguousarray(c2 * (2 * P) + 2 * p)


def kernel(**inputs) -> tuple[np.ndarray, np.ndarray]:
    x = np.ascontiguousarray(np.asarray(inputs["x"], dtype=np.float32))
    tok = np.ascontiguousarray(np.asarray(inputs["mask_token"], dtype=np.float32))
    rv = np.ascontiguousarray(np.asarray(inputs["rand_vals"], dtype=np.float32))
    iota = _iota2()

    in_maps = [
        {
            "x": x[i * B_LOC : (i + 1) * B_LOC].reshape(ROWS, D),
            "rand_vals": rv[i * B_LOC : (i + 1) * B_LOC].reshape(ROWS),
            "mask_token": tok,
            "iota2": iota,
        }
        for i in range(N_CORES)
    ]
    res = run_bass_kernel_spmd(_get_nc(), in_maps, list(range(N_CORES)))
    x_out = np.concatenate(
        [res.results[i]["x_out"].reshape(B_LOC, T, D) for i in range(N_CORES)], axis=0
    )
    mask = np.concatenate(
        [res.results[i]["mask_out"].reshape(B_LOC, T) for i in range(N_CORES)], axis=0
    )
    return x_out, mask


# revision 32
# speedup vs baseline: 1.0775x; 1.0466x over previous
"""Trainium2 Bass kernel for D2VEncoder.mask_tokens (span masking).

Reference semantics (B=16, T=4096, D=768, L=10, p=0.065):
    start[b,t]  = rand_vals[b,t] < p
    masked[b,t] = OR_{k=0..9} start[b, t-k]          (within the batch row)
    x_out       = where(masked[:, :, None], mask_token, x)
    mask        = where(masked, -inf, 0.0)

Sharding: data-parallel over batch, 2 rows per core on 8 NeuronCores; the
mask expansion is elementwise per (b,t) so no collectives are needed.

Per-core dataflow (memory-bound problem; ~49% of (b,t) rows are masked and
their x values are never needed):
  1. Window-OR over start indicators in chunk-major [64,128] layout on DVE
     (9 shifted adds with a 9-element halo loaded by a second small DMA).
  2. Predicate transposed to row-per-partition layout via a DRAM round-trip
     (DMA APs over DRAM are freely affine).
  3. x is streamed in 16 tiles of 512 rows. Each indirect-DMA index gathers
     TWO consecutive rows (6 KB); pairs with both rows masked get an
     out-of-bounds offset, which the DGE silently skips — cutting ~45% of
     read traffic. copy_predicated overwrites every masked row with
     mask_token, which also covers skipped pairs' stale SBUF slots.
     The first DENSE_HEAD tiles are read densely so streaming starts while
     the predicate chain is still computing.
  4. The 0/-inf attention mask is produced from the same predicate.

This compile path (bass2jax / walrus custom-kernel pipeline) accepts at most
ONE sync wait per instruction; _legalize_waits splits Tile's multi-wait
instructions into EventSemaphore + instruction, which is semantically
identical (the sequencer blocks on them in order).
"""

import numpy as np

import concourse.bass as bass
import concourse.mybir as mybir
from concourse.tile import TileContext, add_dep_helper
from concourse.bass_utils import run_bass_kernel_spmd

F32 = mybir.dt.float32
I32 = mybir.dt.int32

B, T, D = 16, 4096, 768
N_CORES = 8
B_LOC = B // N_CORES            # 2 batch rows per core
ROWS = B_LOC * T                # 8192 (b,t) rows per core
P = 128                         # partitions
N_CHUNK = ROWS // P             # 64 chunks of 128 rows
CH_PER_B = T // P               # 32 chunks per batch row
N_CHUNK2 = ROWS // (2 * P)      # 32 blocks of 256 rows (gather granularity)
L = 10                          # span length
HALO = L - 1
P_MASKING = 0.065

K2 = 2                          # 256-row blocks per streamed mega-tile
N_MEGA = N_CHUNK2 // K2         # 16 tiles of 512 rows
STREAM_BUFS = 8
DENSE_HEAD = 2                  # leading tiles read dense (no offsets dep)

_nc_cache = {}


def _legalize_waits(nc: bass.Bass) -> bass.Bass:
    """Split multi-wait instructions into EventSemaphore + instruction."""
    ctr = 0
    for f in nc.m.functions:
        for blk in f.blocks:
            out = []
            for ins in blk.instructions:
                si = ins.sync_info
                if si is not None and si.on_wait and len(si.on_wait) > 1:
                    waits = list(si.on_wait)
                    for w in waits[:-1]:
                        evs = mybir.InstEventSemaphore(ins=[], outs=[])
                        evs.name = f"I-waitsplit-{ctr}"
                        ctr += 1
                        evs.engine = ins.engine
                        evs.sync_info = mybir.SyncInfo(on_wait=[w], on_update=[])
                        out.append(evs)
                    ins.sync_info = mybir.SyncInfo(
                        on_wait=[waits[-1]], on_update=list(si.on_update)
                    )
                out.append(ins)
            blk.instructions[:] = out
    return nc


def _build_nc() -> bass.Bass:
    nc = bass.Bass()
    x = nc.declare_dram_parameter("x", [ROWS, D], F32, isOutput=False)
    rv = nc.declare_dram_parameter("rand_vals", [ROWS], F32, isOutput=False)
    tok = nc.declare_dram_parameter("mask_token", [D], F32, isOutput=False)
    iota2 = nc.declare_dram_parameter("iota2", [P, N_CHUNK2], I32, isOutput=False)
    xo = nc.declare_dram_parameter("x_out", [ROWS, D], F32, isOutput=True)
    mo = nc.declare_dram_parameter("mask_out", [ROWS], F32, isOutput=True)

    with TileContext(nc) as tc:
        with (
            tc.tile_pool(name="const", bufs=1) as cpool,
            tc.tile_pool(name="small", bufs=1) as spool,
            tc.tile_pool(name="dram", bufs=1, space="DRAM") as dpool,
            tc.tile_pool(name="stream", bufs=STREAM_BUFS) as stpool,
        ):
            # mask_token replicated to all 128 partitions
            tokt = cpool.tile([P, D], F32)
            nc.sync.dma_start(tokt[:], tok[None, :].broadcast_to([P, D]))
            # gate: make DVE observe the tokt DMA once so later consumers
            # need no extra wait
            gate_a = spool.tile([1, 1], F32)
            nc.vector.tensor_copy(gate_a[:], tokt[0:1, 0:1])

            # pair-base row indices iota2[p, c2] = c2*256 + 2p
            iott = cpool.tile([P, N_CHUNK2], I32)
            nc.sync.dma_start(iott[:], iota2[:, :])

            # rand values, chunk-major [64, 128]. Prep DMAs ride the scalar
            # (ACT) HWDGE ring so they don't queue behind the stream DMAs.
            rmain = spool.tile([N_CHUNK, P], F32)
            nc.scalar.dma_start(rmain[:], rv[:].rearrange("(c p) -> c p", p=P))
            # 9-element halo: chunk c gets rand[c*128-9 : c*128] (partitions
            # 1..63; rows 0 and 32 are batch-row starts -> no halo)
            rprev = spool.tile([N_CHUNK, HALO], F32)
            nc.scalar.dma_start(
                rprev[1:N_CHUNK, :],
                bass.AP(rv, P - HALO, [[P, N_CHUNK - 1], [1, HALO]]),
            )
            # overwrite the cross-batch-row halo (and chunk 0) with 1.0 (>= p)
            nc.vector.memset(rprev[0:1, :], 1.0)
            for b in range(1, B_LOC):
                nc.vector.memset(rprev[b * CH_PER_B : b * CH_PER_B + 1, :], 1.0)

            # halo'd start indicators, then the 10-wide window sum as a log
            # tree (10 = 8+2): 4 adds instead of 9 serial ones.
            # s01 col 9+j holds start[j] for j in [-9, 127].
            s01 = spool.tile([N_CHUNK, HALO + P], F32)
            nc.vector.tensor_scalar(
                s01[:, 0:HALO], rprev[:], P_MASKING, None, mybir.AluOpType.is_lt
            )
            nc.vector.tensor_scalar(
                s01[:, HALO:], rmain[:], P_MASKING, None, mybir.AluOpType.is_lt
            )
            t2 = spool.tile([N_CHUNK, P + 8], F32)   # col c: j=c-8, window {j, j-1}
            nc.vector.tensor_tensor(
                t2[:], s01[:, 1:], s01[:, 0 : P + 8], mybir.AluOpType.add
            )
            t4 = spool.tile([N_CHUNK, P + 6], F32)   # col c: j=c-6, window 4
            nc.vector.tensor_tensor(
                t4[:], t2[:, 2:], t2[:, 0 : P + 6], mybir.AluOpType.add
            )
            t8 = spool.tile([N_CHUNK, P + 2], F32)   # col c: j=c-2, window 8
            nc.vector.tensor_tensor(
                t8[:], t4[:, 4:], t4[:, 0 : P + 2], mybir.AluOpType.add
            )
            acc = spool.tile([N_CHUNK, P], F32)      # window {j..j-7} + {j-8, j-9}
            nc.vector.tensor_tensor(
                acc[:], t8[:, 2:], t2[:, 0:P], mybir.AluOpType.add
            )
            # integer 0/1 predicate (CopyPredicated requires an int mask)
            acc_i = spool.tile([N_CHUNK, P], I32)
            nc.vector.tensor_scalar(
                acc_i[:], acc[:], 0.0, None, mybir.AluOpType.is_gt
            )

            # pair-AND computed chunk-major BEFORE the round-trip so the
            # gather-critical transposed read is half the bytes and issues
            # first on the scalar ring: pairc[c, u] = pred[c*128+2u] AND +1
            pairc = spool.tile([N_CHUNK, P // 2], I32)
            a3 = acc_i[:].rearrange("c (u e) -> c u e", e=2)
            nc.vector.tensor_tensor(
                pairc[:], a3[:, :, 0], a3[:, :, 1], mybir.AluOpType.mult
            )
            pairD = dpool.tile([ROWS // 2], I32)
            nc.scalar.dma_start(
                pairD[:].rearrange("(c u) -> c u", u=P // 2), pairc[:]
            )
            # pair2T[p, c2] = pairc flat[c2*128 + p]  (pair at rows c2*256+2p)
            pair2T = spool.tile([P, N_CHUNK2], I32)
            nc.scalar.dma_start(
                pair2T[:],
                bass.AP(pairD[:].tensor, pairD[:].offset, [[1, P], [P, N_CHUNK2]]),
            )
            offs2 = spool.tile([P, N_CHUNK2], I32)
            nc.vector.tensor_scalar(
                offs2[:], pair2T[:], ROWS, None, mybir.AluOpType.mult
            )
            offs_done = nc.vector.tensor_tensor(
                offs2[:], offs2[:], iott[:], mybir.AluOpType.add
            )

            # full per-row predicate, pair-transposed, for the CP masks
            # (off the gather critical path):
            # pred2T[p, 2*c2+e] = pred[row c2*256 + 2p + e]
            # Round-tripped in two halves so the first tiles' masks arrive
            # a few us earlier (first CP gates the first x_out write).
            predD = dpool.tile([ROWS], I32)
            pred2T = spool.tile([P, N_CHUNK], I32)
            predC = predD[:].rearrange("(c p) -> c p", p=P)
            for h in range(4):
                ch = N_CHUNK // 4
                nc.scalar.dma_start(
                    predC[h * ch : (h + 1) * ch, :],
                    acc_i[h * ch : (h + 1) * ch, :],
                )
                nc.scalar.dma_start(
                    pred2T[:, h * ch : (h + 1) * ch],
                    bass.AP(
                        predD[:].tensor,
                        predD[:].offset + h * (ROWS // 4),
                        [[2, P], [2 * P, N_CHUNK2 // 4], [1, 2]],
                    ),
                )

            # attention-mask output: 0 kept / -inf masked
            mval = spool.tile([N_CHUNK, P], F32)
            minf = spool.tile([N_CHUNK, P], F32)
            nc.vector.memset(mval[:], 0.0)
            nc.vector.memset(minf[:], float("-inf"))
            nc.vector.copy_predicated(mval[:], acc_i[:], minf[:])
            nc.sync.dma_start(mo[:].rearrange("(c p) -> c p", p=P), mval[:])

            # stream x: gather pairs with >=1 unmasked row, then overwrite
            # all masked rows with the token (covers skipped pairs too).
            # One shared bounds register (a to_reg per gather would exhaust
            # the gpsimd register file). The last mega-tile is split in two
            # so the final (pipeline-tail) store is half as long.
            bc_reg = nc.gpsimd.to_reg(ROWS - 1)
            blocks = [(i * K2, K2) for i in range(N_MEGA - 1)]
            blocks += [((N_MEGA - 1) * K2 + j, 1) for j in range(K2)]
            for bi, (c2base, kk) in enumerate(blocks):
                tile = stpool.tile([P, K2 * 2 * D], F32)
                if c2base < DENSE_HEAD * K2:
                    src = x[c2base * 2 * P : (c2base + kk) * 2 * P, :].rearrange(
                        "(q p e) d -> p q e d", q=kk, e=2
                    )
                    nc.sync.dma_start(
                        tile[:, : kk * 2 * D].rearrange(
                            "p (q e d) -> p q e d", q=kk, e=2
                        ),
                        src,
                    )
                else:
                    for q in range(kk):
                        g = c2base + q
                        nc.gpsimd.indirect_dma_start(
                            out=tile[:, q * 2 * D : (q + 1) * 2 * D],
                            out_offset=None,
                            in_=x[:],
                            in_offset=bass.IndirectOffsetOnAxis(
                                ap=offs2[:, g : g + 1], axis=0
                            ),
                            bounds_check=bc_reg,
                            oob_is_err=False,
                        )
                # per-row selects: fine granularity pipelines best (each CP
                # starts right after its own half-tile's gather; a single
                # merged 3D-broadcast CP measured slower per element AND
                # serialized behind both gathers)
                for q in range(kk):
                    g = c2base + q
                    for e in range(2):
                        mask_ap = pred2T[:, 2 * g + e : 2 * g + e + 1].broadcast_to(
                            [P, D]
                        )
                        cp = nc.vector.copy_predicated(
                            tile[:, (q * 2 + e) * D : (q * 2 + e + 1) * D],
                            mask_ap,
                            tokt[:],
                        )
                        if c2base < DENSE_HEAD * K2:
                            # keep the offsets chain ahead of these selects in
                            # DVE order: the first gather's wait otherwise
                            # trails the dense-head CPs by ~9us
                            add_dep_helper(
                                cp.ins,
                                offs_done.ins,
                                sync=False,
                                reason="offsets chain before dense-head CPs",
                            )
                dst = xo[c2base * 2 * P : (c2base + kk) * 2 * P, :].rearrange(
                    "(q p e) d -> p q e d", q=kk, e=2
                )
                tv = tile[:, : kk * 2 * D].rearrange(
                    "p (q e d) -> p q e d", q=kk, e=2
                )
                # alternate stores across the two HWDGE rings (the scalar
                # ring is idle once the predicate prep finishes)
                (nc.sync if bi % 2 == 0 else nc.scalar).dma_start(dst, tv)

    return _legalize_waits(nc)


def _get_nc() -> bass.Bass:
    if "nc" not in _nc_cache:
        _nc_cache["nc"] = _build_nc()
    return _nc_cache["nc"]


def _iota2() -> np.ndarray:
    p = np.arange(P, dtype=np.int32)[:, None]
    c2 = np.arange(N_CHUNK2, dtype=np.int32)[None, :]
    return np.ascontiguousarray(c2 * (2 * P) + 2 * p)


def kernel(**inputs) -> tuple[np.ndarray, np.ndarray]:
    x = np.ascontiguousarray(np.asarray(inputs["x"], dtype=np.float32))
    tok = np.ascontiguousarray(np.asarray(inputs["mask_token"], dtype=np.float32))
    rv = np.ascontiguousarray(np.asarray(inputs["rand_vals"], dtype=np.float32))
    iota = _iota2()

    in_maps = [
        {
            "x": x[i * B_LOC : (i + 1) * B_LOC].reshape(ROWS, D),
            "rand_vals": rv[i * B_LOC : (i + 1) * B_LOC].reshape(ROWS),
            "mask_token": tok,
            "iota2": iota,
        }
        for i in range(N_CORES)
    ]
    res = run_bass_kernel_spmd(_get_nc(), in_maps, list(range(N_CORES)))
    x_out = np.concatenate(
        [res.results[i]["x_out"].reshape(B_LOC, T, D) for i in range(N_CORES)], axis=0
    )
    mask = np.concatenate(
        [res.results[i]["mask_out"].reshape(B_LOC, T) for i in range(N_CORES)], axis=0
    )
    return x_out, mask


# revision 33
# speedup vs baseline: 1.0909x; 1.0125x over previous
"""Trainium2 Bass kernel for D2VEncoder.mask_tokens (span masking).

Reference semantics (B=16, T=4096, D=768, L=10, p=0.065):
    start[b,t]  = rand_vals[b,t] < p
    masked[b,t] = OR_{k=0..9} start[b, t-k]          (within the batch row)
    x_out       = where(masked[:, :, None], mask_token, x)
    mask        = where(masked, -inf, 0.0)

Sharding: data-parallel over batch, 2 rows per core on 8 NeuronCores; the
mask expansion is elementwise per (b,t) so no collectives are needed.

Per-core dataflow (memory-bound problem; ~49% of (b,t) rows are masked and
their x values are never needed):
  1. Window-OR over start indicators in chunk-major [64,128] layout on DVE
     (9 shifted adds with a 9-element halo loaded by a second small DMA).
  2. Predicate transposed to row-per-partition layout via a DRAM round-trip
     (DMA APs over DRAM are freely affine).
  3. x is streamed in 16 tiles of 512 rows. Each indirect-DMA index gathers
     TWO consecutive rows (6 KB); pairs with both rows masked get an
     out-of-bounds offset, which the DGE silently skips — cutting ~45% of
     read traffic. copy_predicated overwrites every masked row with
     mask_token, which also covers skipped pairs' stale SBUF slots.
     The first DENSE_HEAD tiles are read densely so streaming starts while
     the predicate chain is still computing.
  4. The 0/-inf attention mask is produced from the same predicate.

This compile path (bass2jax / walrus custom-kernel pipeline) accepts at most
ONE sync wait per instruction; _legalize_waits splits Tile's multi-wait
instructions into EventSemaphore + instruction, which is semantically
identical (the sequencer blocks on them in order).
"""

import numpy as np

import concourse.bass as bass
import concourse.mybir as mybir
from concourse.tile import TileContext, add_dep_helper
from concourse.bass_utils import run_bass_kernel_spmd

F32 = mybir.dt.float32
I32 = mybir.dt.int32

B, T, D = 16, 4096, 768
N_CORES = 8
B_LOC = B // N_CORES            # 2 batch rows per core
ROWS = B_LOC * T                # 8192 (b,t) rows per core
P = 128                         # partitions
N_CHUNK = ROWS // P             # 64 chunks of 128 rows
CH_PER_B = T // P               # 32 chunks per batch row
N_CHUNK2 = ROWS // (2 * P)      # 32 blocks of 256 rows (gather granularity)
L = 10                          # span length
HALO = L - 1
P_MASKING = 0.065

K2 = 2                          # 256-row blocks per streamed mega-tile
N_MEGA = N_CHUNK2 // K2         # 16 tiles of 512 rows
STREAM_BUFS = 8
DENSE_HEAD = 3                  # leading tiles read dense (no offsets dep)

_nc_cache = {}


def _legalize_waits(nc: bass.Bass) -> bass.Bass:
    """Split multi-wait instructions into EventSemaphore + instruction."""
    ctr = 0
    for f in nc.m.functions:
        for blk in f.blocks:
            out = []
            for ins in blk.instructions:
                si = ins.sync_info
                if si is not None and si.on_wait and len(si.on_wait) > 1:
                    waits = list(si.on_wait)
                    for w in waits[:-1]:
                        evs = mybir.InstEventSemaphore(ins=[], outs=[])
                        evs.name = f"I-waitsplit-{ctr}"
                        ctr += 1
                        evs.engine = ins.engine
                        evs.sync_info = mybir.SyncInfo(on_wait=[w], on_update=[])
                        out.append(evs)
                    ins.sync_info = mybir.SyncInfo(
                        on_wait=[waits[-1]], on_update=list(si.on_update)
                    )
                out.append(ins)
            blk.instructions[:] = out
    return nc


def _build_nc() -> bass.Bass:
    nc = bass.Bass()
    x = nc.declare_dram_parameter("x", [ROWS, D], F32, isOutput=False)
    rv = nc.declare_dram_parameter("rand_vals", [ROWS], F32, isOutput=False)
    tok = nc.declare_dram_parameter("mask_token", [D], F32, isOutput=False)
    iota2 = nc.declare_dram_parameter("iota2", [P, N_CHUNK2], I32, isOutput=False)
    xo = nc.declare_dram_parameter("x_out", [ROWS, D], F32, isOutput=True)
    mo = nc.declare_dram_parameter("mask_out", [ROWS], F32, isOutput=True)

    with TileContext(nc) as tc:
        with (
            tc.tile_pool(name="const", bufs=1) as cpool,
            tc.tile_pool(name="small", bufs=1) as spool,
            tc.tile_pool(name="dram", bufs=1, space="DRAM") as dpool,
            tc.tile_pool(name="stream", bufs=STREAM_BUFS) as stpool,
        ):
            # mask_token replicated to all 128 partitions
            tokt = cpool.tile([P, D], F32)
            nc.sync.dma_start(tokt[:], tok[None, :].broadcast_to([P, D]))
            # gate: make DVE observe the tokt DMA once so later consumers
            # need no extra wait
            gate_a = spool.tile([1, 1], F32)
            nc.vector.tensor_copy(gate_a[:], tokt[0:1, 0:1])

            # pair-base row indices iota2[p, c2] = c2*256 + 2p
            iott = cpool.tile([P, N_CHUNK2], I32)
            nc.sync.dma_start(iott[:], iota2[:, :])

            # rand values, chunk-major [64, 128]. Prep DMAs ride the scalar
            # (ACT) HWDGE ring so they don't queue behind the stream DMAs.
            rmain = spool.tile([N_CHUNK, P], F32)
            nc.scalar.dma_start(rmain[:], rv[:].rearrange("(c p) -> c p", p=P))
            # 9-element halo: chunk c gets rand[c*128-9 : c*128] (partitions
            # 1..63; rows 0 and 32 are batch-row starts -> no halo)
            rprev = spool.tile([N_CHUNK, HALO], F32)
            nc.scalar.dma_start(
                rprev[1:N_CHUNK, :],
                bass.AP(rv, P - HALO, [[P, N_CHUNK - 1], [1, HALO]]),
            )
            # overwrite the cross-batch-row halo (and chunk 0) with 1.0 (>= p)
            nc.vector.memset(rprev[0:1, :], 1.0)
            for b in range(1, B_LOC):
                nc.vector.memset(rprev[b * CH_PER_B : b * CH_PER_B + 1, :], 1.0)

            # halo'd start indicators, then the 10-wide window sum as a log
            # tree (10 = 8+2): 4 adds instead of 9 serial ones.
            # s01 col 9+j holds start[j] for j in [-9, 127].
            s01 = spool.tile([N_CHUNK, HALO + P], F32)
            nc.vector.tensor_scalar(
                s01[:, 0:HALO], rprev[:], P_MASKING, None, mybir.AluOpType.is_lt
            )
            nc.vector.tensor_scalar(
                s01[:, HALO:], rmain[:], P_MASKING, None, mybir.AluOpType.is_lt
            )
            t2 = spool.tile([N_CHUNK, P + 8], F32)   # col c: j=c-8, window {j, j-1}
            nc.vector.tensor_tensor(
                t2[:], s01[:, 1:], s01[:, 0 : P + 8], mybir.AluOpType.add
            )
            t4 = spool.tile([N_CHUNK, P + 6], F32)   # col c: j=c-6, window 4
            nc.vector.tensor_tensor(
                t4[:], t2[:, 2:], t2[:, 0 : P + 6], mybir.AluOpType.add
            )
            t8 = spool.tile([N_CHUNK, P + 2], F32)   # col c: j=c-2, window 8
            nc.vector.tensor_tensor(
                t8[:], t4[:, 4:], t4[:, 0 : P + 2], mybir.AluOpType.add
            )
            acc = spool.tile([N_CHUNK, P], F32)      # window {j..j-7} + {j-8, j-9}
            nc.vector.tensor_tensor(
                acc[:], t8[:, 2:], t2[:, 0:P], mybir.AluOpType.add
            )
            # integer 0/1 predicate (CopyPredicated requires an int mask)
            acc_i = spool.tile([N_CHUNK, P], I32)
            nc.vector.tensor_scalar(
                acc_i[:], acc[:], 0.0, None, mybir.AluOpType.is_gt
            )

            # pair-AND computed chunk-major BEFORE the round-trip so the
            # gather-critical transposed read is half the bytes and issues
            # first on the scalar ring: pairc[c, u] = pred[c*128+2u] AND +1
            pairc = spool.tile([N_CHUNK, P // 2], I32)
            a3 = acc_i[:].rearrange("c (u e) -> c u e", e=2)
            nc.vector.tensor_tensor(
                pairc[:], a3[:, :, 0], a3[:, :, 1], mybir.AluOpType.mult
            )
            pairD = dpool.tile([ROWS // 2], I32)
            nc.scalar.dma_start(
                pairD[:].rearrange("(c u) -> c u", u=P // 2), pairc[:]
            )
            # pair2T[p, c2] = pairc flat[c2*128 + p]  (pair at rows c2*256+2p)
            pair2T = spool.tile([P, N_CHUNK2], I32)
            nc.scalar.dma_start(
                pair2T[:],
                bass.AP(pairD[:].tensor, pairD[:].offset, [[1, P], [P, N_CHUNK2]]),
            )
            offs2 = spool.tile([P, N_CHUNK2], I32)
            nc.vector.tensor_scalar(
                offs2[:], pair2T[:], ROWS, None, mybir.AluOpType.mult
            )
            offs_done = nc.vector.tensor_tensor(
                offs2[:], offs2[:], iott[:], mybir.AluOpType.add
            )

            # full per-row predicate, pair-transposed, for the CP masks
            # (off the gather critical path):
            # pred2T[p, 2*c2+e] = pred[row c2*256 + 2p + e]
            # Round-tripped in two halves so the first tiles' masks arrive
            # a few us earlier (first CP gates the first x_out write).
            predD = dpool.tile([ROWS], I32)
            pred2T = spool.tile([P, N_CHUNK], I32)
            predC = predD[:].rearrange("(c p) -> c p", p=P)
            for h in range(4):
                ch = N_CHUNK // 4
                nc.scalar.dma_start(
                    predC[h * ch : (h + 1) * ch, :],
                    acc_i[h * ch : (h + 1) * ch, :],
                )
                nc.scalar.dma_start(
                    pred2T[:, h * ch : (h + 1) * ch],
                    bass.AP(
                        predD[:].tensor,
                        predD[:].offset + h * (ROWS // 4),
                        [[2, P], [2 * P, N_CHUNK2 // 4], [1, 2]],
                    ),
                )

            # attention-mask output: 0 kept / -inf masked
            mval = spool.tile([N_CHUNK, P], F32)
            minf = spool.tile([N_CHUNK, P], F32)
            nc.vector.memset(mval[:], 0.0)
            nc.vector.memset(minf[:], float("-inf"))
            nc.vector.copy_predicated(mval[:], acc_i[:], minf[:])
            nc.sync.dma_start(mo[:].rearrange("(c p) -> c p", p=P), mval[:])

            # stream x: gather pairs with >=1 unmasked row, then overwrite
            # all masked rows with the token (covers skipped pairs too).
            # One shared bounds register (a to_reg per gather would exhaust
            # the gpsimd register file). The last mega-tile is split in two
            # so the final (pipeline-tail) store is half as long.
            bc_reg = nc.gpsimd.to_reg(ROWS - 1)
            blocks = [(i * K2, K2) for i in range(N_MEGA - 1)]
            blocks += [((N_MEGA - 1) * K2 + j, 1) for j in range(K2)]
            for bi, (c2base, kk) in enumerate(blocks):
                tile = stpool.tile([P, K2 * 2 * D], F32)
                if c2base < DENSE_HEAD * K2:
                    src = x[c2base * 2 * P : (c2base + kk) * 2 * P, :].rearrange(
                        "(q p e) d -> p q e d", q=kk, e=2
                    )
                    nc.sync.dma_start(
                        tile[:, : kk * 2 * D].rearrange(
                            "p (q e d) -> p q e d", q=kk, e=2
                        ),
                        src,
                    )
                else:
                    for q in range(kk):
                        g = c2base + q
                        nc.gpsimd.indirect_dma_start(
                            out=tile[:, q * 2 * D : (q + 1) * 2 * D],
                            out_offset=None,
                            in_=x[:],
                            in_offset=bass.IndirectOffsetOnAxis(
                                ap=offs2[:, g : g + 1], axis=0
                            ),
                            bounds_check=bc_reg,
                            oob_is_err=False,
                        )
                # per-row selects: fine granularity pipelines best (each CP
                # starts right after its own half-tile's gather; a single
                # merged 3D-broadcast CP measured slower per element AND
                # serialized behind both gathers)
                for q in range(kk):
                    g = c2base + q
                    for e in range(2):
                        mask_ap = pred2T[:, 2 * g + e : 2 * g + e + 1].broadcast_to(
                            [P, D]
                        )
                        cp = nc.vector.copy_predicated(
                            tile[:, (q * 2 + e) * D : (q * 2 + e + 1) * D],
                            mask_ap,
                            tokt[:],
                        )
                        if c2base < DENSE_HEAD * K2:
                            # keep the offsets chain ahead of these selects in
                            # DVE order: the first gather's wait otherwise
                            # trails the dense-head CPs by ~9us
                            add_dep_helper(
                                cp.ins,
                                offs_done.ins,
                                sync=False,
                                reason="offsets chain before dense-head CPs",
                            )
                dst = xo[c2base * 2 * P : (c2base + kk) * 2 * P, :].rearrange(
                    "(q p e) d -> p q e d", q=kk, e=2
                )
                tv = tile[:, : kk * 2 * D].rearrange(
                    "p (q e d) -> p q e d", q=kk, e=2
                )
                # alternate stores across the two HWDGE rings (the scalar
                # ring is idle once the predicate prep finishes)
                (nc.sync if bi % 2 == 0 else nc.scalar).dma_start(dst, tv)

    return _legalize_waits(nc)


def _get_nc() -> bass.Bass:
    if "nc" not in _nc_cache:
        _nc_cache["nc"] = _build_nc()
    return _nc_cache["nc"]


def _iota2() -> np.ndarray:
    p = np.arange(P, dtype=np.int32)[:, None]
    c2 = np.arange(N_CHUNK2, dtype=np.int32)[None, :]
    return np.ascontiguousarray(c2 * (2 * P) + 2 * p)


def kernel(**inputs) -> tuple[np.ndarray, np.ndarray]:
    x = np.ascontiguousarray(np.asarray(inputs["x"], dtype=np.float32))
    tok = np.ascontiguousarray(np.asarray(inputs["mask_token"], dtype=np.float32))
    rv = np.ascontiguousarray(np.asarray(inputs["rand_vals"], dtype=np.float32))
    iota = _iota2()

    in_maps = [
        {
            "x": x[i * B_LOC : (i + 1) * B_LOC].reshape(ROWS, D),
            "rand_vals": rv[i * B_LOC : (i + 1) * B_LOC].reshape(ROWS),
            "mask_token": tok,
            "iota2": iota,
        }
        for i in range(N_CORES)
    ]
    res = run_bass_kernel_spmd(_get_nc(), in_maps, list(range(N_CORES)))
    x_out = np.concatenate(
        [res.results[i]["x_out"].reshape(B_LOC, T, D) for i in range(N_CORES)], axis=0
    )
    mask = np.concatenate(
        [res.results[i]["mask_out"].reshape(B_LOC, T) for i in range(N_CORES)], axis=0
    )
    return x_out, mask
